# revision 29
# baseline (speedup 1.0000x reference)
"""Trainium2 Bass kernel for nn_C2f_DualModal_MoE (C2f block with top-1 MoE routing).

Strategy (data-parallel over batch, 4 samples per core on 8 cores):
  - cv1 (1x1 conv 256->256 + SiLU) as f32r matmuls over 400-pixel tiles;
    the `feat` half is written into a zero-padded [82x82] spatial layout so
    the 3x3 convs become 9 shift-offset matmuls. The global-average-pool for
    the router comes free via the activation accum_out.
  - Router: tiny f32 matmul + softmax on-chip; the top-1 selection is turned
    into a one-hot vector (no control flow), which selects the routed expert's
    weights via 3 vector ops (Wsel = sum_e onehot[e] * We[e]); since top-1,
    conv(feat, Wsel) == conv(feat, We[argmax]).
  - shared + routed 3x3 convs (SiLU), moe = shared + gate * routed.
  - cv2 (1x1 conv 384->256 + SiLU) fused per tile from (a, feat, moe) without
    materializing the concat; routed-conv and cv2 are software-pipelined by
    one tile.
All matmuls use float32r (full-rate PE); everything else f32.
"""

import numpy as np

import concourse.bass as bass
import concourse.bacc as bacc
import concourse.tile as tile
from concourse import bass_isa, mybir
from concourse.bass_utils import run_bass_kernel_spmd

# Problem constants (hardcoded per contract)
B, C1, C2 = 32, 256, 256
H = W = 80
CH = 128
NE = 3
NCORES = 8
BPC = B // NCORES          # samples per core = 4
NPIX = H * W               # 6400
PADW = W + 2               # 82
PADH = H + 2               # 82
RPT = 5                    # rows per pixel tile
TN = RPT * W               # 400 pixels per tile
NT = H // RPT              # 16 tiles
NP = NT // 2               # 8 tile-pairs
TAPS = [(dy, dx) for dy in range(3) for dx in range(3)]

f32 = mybir.dt.float32
f32r = mybir.dt.float32r
bf16 = mybir.dt.bfloat16


def _emit(nc, tc, ctx, reps=1, sim_compat=False, tune=None, internal_io=False):
    AX = mybir.AxisListType
    OP = mybir.AluOpType
    AF = mybir.ActivationFunctionType
    tune = {**dict(xbufs=4, obufs=4, rbufs=2, psbufs=3, fpdouble=False,
                   adouble=False, bf16=False, g2=False, cv2co=False,
                   adefer=False, psmerge=False, ybf16=False,
                   shbf16=False, gsrouter=False, amerge=False), **(tune or {})}
    dmm = bf16 if tune["bf16"] else f32r
    ydt = bf16 if tune["ybf16"] else f32

    io_kind = "Internal" if internal_io else "ExternalInput"
    x_d = nc.dram_tensor("x", [BPC, 2, CH, NPIX], dmm, kind=io_kind).ap()
    w1_d = nc.dram_tensor("w1t", [2, CH, 2 * CH], dmm, kind="ExternalInput").ap()
    b1_d = nc.dram_tensor("b1r", [2, CH], f32, kind="ExternalInput").ap()
    wr_d = nc.dram_tensor("wrs", [CH, NE], f32, kind="ExternalInput").ap()
    br_d = nc.dram_tensor("brr", [CH, NE], f32, kind="ExternalInput").ap()
    ws_d = nc.dram_tensor("wst", [CH, 9 * CH], dmm, kind="ExternalInput").ap()
    bs_d = nc.dram_tensor("bsr", [CH, 1], f32, kind="ExternalInput").ap()
    we_d = nc.dram_tensor("wet", [NE, CH, 9 * CH], f32, kind="ExternalInput").ap()
    be_d = nc.dram_tensor("ber", [CH, NE], f32, kind="ExternalInput").ap()
    w2_d = nc.dram_tensor("w2t", [3, CH, C2], dmm, kind="ExternalInput").ap()
    b2_d = nc.dram_tensor("b2r", [2, CH], f32, kind="ExternalInput").ap()
    y_d = nc.dram_tensor(
        "y", [BPC, 2, CH, NPIX], ydt,
        kind="Internal" if internal_io else "ExternalOutput").ap()

    wpool = ctx.enter_context(tc.tile_pool(name="weights", bufs=1))
    ppool = ctx.enter_context(tc.tile_pool(name="persist", bufs=1))
    xpool = ctx.enter_context(tc.tile_pool(name="xin", bufs=tune["xbufs"]))
    opool = ctx.enter_context(tc.tile_pool(name="oout", bufs=tune["obufs"]))
    rpool = ctx.enter_context(tc.tile_pool(name="rtile", bufs=tune["rbufs"]))
    spool = ctx.enter_context(tc.tile_pool(name="small", bufs=2))
    selpool = ctx.enter_context(tc.tile_pool(name="sel", bufs=1))
    psum = ctx.enter_context(tc.tile_pool(name="psum", bufs=tune["psbufs"], space="PSUM"))
    psumS = psum if tune["psmerge"] else ctx.enter_context(
        tc.tile_pool(name="psumS", bufs=1, space="PSUM"))

    # ---- load weights into SBUF (resident) ----
    w1_sb = wpool.tile([CH, 2 * 2 * CH], dmm)
    for k in range(2):
        nc.sync.dma_start(w1_sb[:, k * 256:(k + 1) * 256], w1_d[k])
    ws_sb = wpool.tile([CH, 9 * CH], dmm)
    nc.sync.dma_start(ws_sb[:], ws_d)
    we_sb = wpool.tile([CH, NE * 9 * CH], f32)
    for e in range(NE):
        nc.sync.dma_start(we_sb[:, e * 1152:(e + 1) * 1152], we_d[e])
    w2_sb = wpool.tile([CH, 3 * C2], dmm)
    for k in range(3):
        nc.sync.dma_start(w2_sb[:, k * 256:(k + 1) * 256], w2_d[k])
    wr_sb = wpool.tile([CH, NE], f32)
    nc.sync.dma_start(wr_sb[:], wr_d)
    if tune["gsrouter"]:
        br_sb = wpool.tile([CH, NE], f32, name="br_sb")
        nc.sync.dma_start(br_sb[:], br_d)
    else:
        br_sb = wpool.tile([1, NE], f32, name="br_sb")
        nc.sync.dma_start(br_sb[:], br_d[0:1, :])
    bs_sb = wpool.tile([CH, 1], f32)
    nc.sync.dma_start(bs_sb[:], bs_d)
    be_sb = wpool.tile([CH, NE], f32)
    nc.sync.dma_start(be_sb[:], be_d)
    b1_sb = wpool.tile([CH, 2], f32)
    for k in range(2):
        nc.sync.dma_start(b1_sb[:, k:k + 1], b1_d[k])
    b2_sb = wpool.tile([CH, 2], f32)
    for k in range(2):
        nc.sync.dma_start(b2_sb[:, k:k + 1], b2_d[k])
    ones_sb = wpool.tile([1, CH], f32)
    nc.vector.memset(ones_sb[:], 1.0)

    if internal_io:
        # timing mode: x is Internal (uninitialized) DRAM; zero it once so
        # the timed loop computes on deterministic, non-denormal data.
        zs = wpool.tile([CH, 800], dmm, name="zs")
        if tune["bf16"]:
            nc.vector.memset(zs[:], 0.0)
        else:
            nc.vector.memset(zs[:].bitcast(f32), 0.0)
        for zb in range(BPC):
            for zk in range(2):
                for zj in range(NPIX // 800):
                    nc.sync.dma_start(
                        x_d[zb, zk, :, zj * 800:(zj + 1) * 800], zs[:])

    # ---- persistent per-sample working buffers ----
    # (optionally double-buffered across samples to decouple next-sample cv1
    # writes from current-sample conv/cv2 reads)
    fps = []
    for fi in range(2 if tune["fpdouble"] else 1):
        fp = ppool.tile([CH, PADH * PADW], dmm, tag=f"fp{fi}", name=f"fp{fi}")
        # zero once: borders stay zero forever (bitcast: memset lacks f32r)
        if tune["bf16"]:
            nc.vector.memset(fp[:], 0.0)
        else:
            nc.vector.memset(fp[:].bitcast(f32), 0.0)
        fps.append(fp[:].rearrange("p (r c) -> p r c", c=PADW))
    a_sbs = [ppool.tile([CH, NPIX], dmm, tag=f"a{ai}", name=f"a{ai}")
             for ai in range(2 if tune["adouble"] else 1)]
    sh_sb = ppool.tile([CH, NPIX], dmm if tune["shbf16"] else f32)
    moe_sb = ppool.tile([CH, NPIX], dmm)

    tmpool = ctx.enter_context(tc.tile_pool(name="silutmp", bufs=2)) if sim_compat else None

    def act_silu(out_ap, ps_ap, bias_ap, accum_ap=None):
        """SiLU from PSUM -> SBUF. On HW, one ACT instruction (with optional
        free GAP accumulation). CoreSim lacks Silu, so sim_compat emulates via
        Sigmoid + (ps+bias)*sig, and computes the accumulation separately."""
        if not sim_compat:
            if accum_ap is not None:
                nc.scalar.activation(out_ap, ps_ap, AF.Silu, bias=bias_ap,
                                     scale=1.0, accum_out=accum_ap)
            else:
                nc.scalar.activation(out_ap, ps_ap, AF.Silu, bias=bias_ap,
                                     scale=1.0)
            return
        shp = list(out_ap.shape[1:])
        fs = 1
        for d in shp:
            fs *= d
        tmp = tmpool.tile([CH, 2 * TN], f32, tag="sigmoid_tmp")
        tv = tmp[:, 0:fs]
        if len(shp) == 2:
            tv = tv.rearrange("p (g c) -> p g c", g=shp[0])
        elif len(shp) == 3:
            tv = tv.rearrange("p (g r c) -> p g r c", g=shp[0], r=shp[1])
        nc.scalar.activation(tv, ps_ap, AF.Sigmoid, bias=bias_ap, scale=1.0)
        nc.vector.scalar_tensor_tensor(out_ap, ps_ap, bias_ap, tv,
                                       op0=OP.add, op1=OP.mult)
        if accum_ap is not None:
            axis = [None, AX.X, AX.XY, AX.XYZ][len(shp)]
            nc.vector.reduce_sum(accum_ap, out_ap, axis=axis)

    def conv_tile_matmuls(ps, wsb, i, fp3):
        for t, (dy, dx) in enumerate(TAPS):
            rhs = fp3[:, i * RPT + dy: i * RPT + dy + RPT, dx: dx + W]
            nc.tensor.matmul(
                ps[:],
                wsb[:, t * CH:(t + 1) * CH],
                rhs,
                start=(t == 0),
                stop=(t == 8),
            )

    def _body():
        for b in range(BPC):
            fp3 = fps[b % len(fps)]
            fp3v = fp3  # [128, 82, 82] padded view
            a_sb = a_sbs[b % len(a_sbs)]
            # ---- cv1 over tile-PAIRS: 800 px per ACT, shared-weight MM runs,
            # GAP accumulated for free ----
            gap_sb = spool.tile([CH, NP], f32, tag="gap")
            for pi in range(NP):
                i0 = 2 * pi
                xt0 = xpool.tile([CH, 2 * TN], dmm, tag="xt0")
                nc.sync.dma_start(xt0[:], x_d[b, 0, :, pi * 800:(pi + 1) * 800])
                xt1 = xpool.tile([CH, 2 * TN], dmm, tag="xt1")
                nc.sync.dma_start(xt1[:], x_d[b, 1, :, pi * 800:(pi + 1) * 800])
                ps_a = psum.tile([CH, 2, 512], f32, tag="ps")
                ps_f = psum.tile([CH, 2, 512], f32, tag="ps")
                for k, xt in ((0, xt0), (1, xt1)):
                    for hw_, ps2 in ((0, ps_a), (1, ps_f)):
                        wsl = w1_sb[:, k * 256 + hw_ * 128: k * 256 + hw_ * 128 + 128]
                        for ii in range(2):
                            nc.tensor.matmul(ps2[:, ii, 0:TN], wsl,
                                             xt[:, ii * TN:(ii + 1) * TN],
                                             start=(k == 0), stop=(k == 1))
                a_out = a_sb[:, i0 * TN:(i0 + 2) * TN].rearrange(
                    "p (g c) -> p g c", g=2)
                if tune["adefer"]:
                    # drain a-half raw on DVE; SiLU applied in-place later
                    # (during the shared-conv phase) when ACT has slack.
                    nc.vector.tensor_copy(a_out, ps_a[:, :, 0:TN])
                else:
                    act_silu(a_out, ps_a[:, :, 0:TN], b1_sb[:, 0:1])
                fout = fp3v[:, 1 + 10 * pi: 11 + 10 * pi, 1:1 + W].rearrange(
                    "p (g r) c -> p g r c", g=2)
                act_silu(fout,
                         ps_f[:, :, 0:TN].rearrange("p g (r c) -> p g r c", c=W),
                         b1_sb[:, 1:2], accum_ap=gap_sb[:, pi:pi + 1])

            # ---- router: logits -> softmax -> top-1 one-hot + gate ----
            pooled = spool.tile([CH, 1], f32, tag="pooled")
            nc.vector.reduce_sum(pooled[:], gap_sb[:], axis=AX.X)
            sc = spool.tile([CH, NE + 1], f32, tag="sc")
            if tune["gsrouter"]:
                # PE-free router: per-partition products, gpsimd all-reduce
                # across partitions, then the whole softmax/top-1 chain runs
                # replicated on all 128 partitions (no broadcast needed).
                prod = spool.tile([CH, NE], f32, tag="prod")
                nc.vector.tensor_scalar_mul(prod[:], wr_sb[:], pooled[:, 0:1])
                lg = spool.tile([CH, NE], f32, tag="lg")
                nc.gpsimd.partition_all_reduce(lg[:], prod[:], channels=CH,
                                               reduce_op=bass_isa.ReduceOp.add)
                logits = spool.tile([CH, NE], f32, tag="logits")
                nc.vector.tensor_add(logits[:], lg[:], br_sb[:])
                m_sb = spool.tile([CH, 1], f32, tag="m")
                nc.vector.reduce_max(m_sb[:], logits[:], axis=AX.X)
                negm = spool.tile([CH, 1], f32, tag="negm")
                nc.vector.tensor_scalar_mul(negm[:], m_sb[:], -1.0)
                e_sb = spool.tile([CH, NE], f32, tag="esb")
                nc.scalar.activation(e_sb[:], logits[:], AF.Exp, bias=negm[:],
                                     scale=1.0)
                s_sb = spool.tile([CH, 1], f32, tag="ssb")
                nc.vector.reduce_sum(s_sb[:], e_sb[:], axis=AX.X)
                nc.vector.reciprocal(sc[:, NE:NE + 1], s_sb[:])
                nc.vector.tensor_scalar(sc[:, 0:NE], logits[:], m_sb[:], None,
                                        op0=OP.is_ge)
            else:
                ps_tag = "ps" if tune["psmerge"] else "psl"
                ps_l = psumS.tile([1, NE], f32, tag=ps_tag)
                # wr is pre-scaled by 1/NPIX on the host, so sums work.
                nc.tensor.matmul(ps_l[:], pooled[:], wr_sb[:], start=True,
                                 stop=True)
                logits = spool.tile([1, NE], f32, tag="logits")
                nc.vector.tensor_add(logits[:], ps_l[:], br_sb[:])
                m_sb = spool.tile([1, 1], f32, tag="m")
                nc.vector.reduce_max(m_sb[:], logits[:], axis=AX.X)
                negm = spool.tile([1, 1], f32, tag="negm")
                nc.vector.tensor_scalar_mul(negm[:], m_sb[:], -1.0)
                e_sb = spool.tile([1, NE], f32, tag="esb")
                nc.scalar.activation(e_sb[:], logits[:], AF.Exp, bias=negm[:],
                                     scale=1.0)
                s_sb = spool.tile([1, 1], f32, tag="ssb")
                nc.vector.reduce_sum(s_sb[:], e_sb[:], axis=AX.X)
                wgt = spool.tile([1, 1], f32, tag="wgt")
                nc.vector.reciprocal(wgt[:], s_sb[:])
                oh = spool.tile([1, NE], f32, tag="oh")
                nc.vector.tensor_scalar(oh[:], logits[:], m_sb[:], None,
                                        op0=OP.is_ge)
                bc = spool.tile([1, NE + 1], f32, tag="bc")
                nc.vector.tensor_copy(bc[:, 0:NE], oh[:])
                nc.vector.tensor_copy(bc[:, NE:NE + 1], wgt[:])
                ps_bc = psumS.tile([CH, NE + 1], f32,
                                   tag="ps" if tune["psmerge"] else "psb")
                nc.tensor.matmul(ps_bc[:], ones_sb[:], bc[:], start=True,
                                 stop=True)
                nc.vector.tensor_copy(sc[:], ps_bc[:])

            # ---- expert-weight select: Wsel = sum_e onehot[e] * We[e] ----
            wA = selpool.tile([CH, 9 * CH], f32, tag="wA")
            nc.vector.tensor_scalar_mul(wA[:], we_sb[:, 0:1152], sc[:, 0:1])
            wB = selpool.tile([CH, 9 * CH], f32, tag="wB")
            nc.vector.scalar_tensor_tensor(wB[:], we_sb[:, 1152:2304], sc[:, 1:2],
                                           wA[:], op0=OP.mult, op1=OP.add)
            wS = selpool.tile([CH, 9 * CH], dmm, tag="wS")
            nc.vector.scalar_tensor_tensor(wS[:], we_sb[:, 2304:3456], sc[:, 2:3],
                                           wB[:], op0=OP.mult, op1=OP.add)
            bA = spool.tile([CH, 1], f32, tag="bA")
            nc.vector.tensor_scalar_mul(bA[:], be_sb[:, 0:1], sc[:, 0:1])
            bB = spool.tile([CH, 1], f32, tag="bB")
            nc.vector.scalar_tensor_tensor(bB[:], be_sb[:, 1:2], sc[:, 1:2],
                                           bA[:], op0=OP.mult, op1=OP.add)
            bS = spool.tile([CH, 1], f32, tag="bS")
            nc.vector.scalar_tensor_tensor(bS[:], be_sb[:, 2:3], sc[:, 2:3],
                                           bB[:], op0=OP.mult, op1=OP.add)

            def conv_pair(ps2, wsb, pi):
                i0 = 2 * pi
                for t, (dy, dx) in enumerate(TAPS):
                    wt = wsb[:, t * CH:(t + 1) * CH]
                    for ii in range(2):
                        rhs = fp3[:, (i0 + ii) * RPT + dy: (i0 + ii) * RPT + dy + RPT,
                                  dx: dx + W]
                        nc.tensor.matmul(ps2[:, ii, 0:TN], wt, rhs,
                                         start=(t == 0), stop=(t == 8))

            def conv_group(wsb, pg):
                # tap-outer over a group of 2 pair-tiles: each tap's weights
                # load once and feed 4 matmuls (2 pairs x 2 half-tiles)
                psA = psum.tile([CH, 2, 512], f32, tag="ps", name="psA")
                psB = psum.tile([CH, 2, 512], f32, tag="ps", name="psB")
                for t, (dy, dx) in enumerate(TAPS):
                    wt = wsb[:, t * CH:(t + 1) * CH]
                    for ps2, pi in ((psA, 2 * pg), (psB, 2 * pg + 1)):
                        i0 = 2 * pi
                        for ii in range(2):
                            rhs = fp3[:, (i0 + ii) * RPT + dy:
                                      (i0 + ii) * RPT + dy + RPT, dx: dx + W]
                            nc.tensor.matmul(ps2[:, ii, 0:TN], wt, rhs,
                                             start=(t == 0), stop=(t == 8))
                return psA, psB

            def a_silu_deferred(pi, npair=1):
                i0 = 2 * pi
                av = a_sb[:, i0 * TN:(i0 + 2 * npair) * TN].rearrange(
                    "p (g c) -> p g c", g=2 * npair)
                nc.scalar.activation(av, av, AF.Silu, bias=b1_sb[:, 0:1],
                                     scale=1.0)

            # ---- shared expert 3x3 conv + SiLU ----
            if tune["g2"]:
                for pg in range(NP // 2):
                    psA, psB = conv_group(ws_sb, pg)
                    for ps2, pi in ((psA, 2 * pg), (psB, 2 * pg + 1)):
                        act_silu(sh_sb[:, pi * 800:(pi + 1) * 800].rearrange(
                                     "p (g c) -> p g c", g=2),
                                 ps2[:, :, 0:TN], bs_sb[:])
                    if tune["adefer"]:
                        a_silu_deferred(2 * pg)
                        a_silu_deferred(2 * pg + 1)
            else:
                for pi in range(NP):
                    ps2 = psum.tile([CH, 2, 512], f32, tag="ps")
                    conv_pair(ps2, ws_sb, pi)
                    act_silu(sh_sb[:, pi * 800:(pi + 1) * 800].rearrange(
                                 "p (g c) -> p g c", g=2),
                             ps2[:, :, 0:TN], bs_sb[:])
                    if tune["adefer"]:
                        if tune["amerge"]:
                            if pi % 2 == 1:
                                a_silu_deferred(pi - 1, npair=2)
                        else:
                            a_silu_deferred(pi)

            # ---- routed conv + moe + fused cv2, software-pipelined ----
            CV2_CHUNKS = (
                (0, lambda i: a_sb[:, i * TN:(i + 1) * TN]),
                (1, lambda i: fp3[:, i * RPT + 1: i * RPT + 1 + RPT, 1: 1 + W]),
                (2, lambda i: moe_sb[:, i * TN:(i + 1) * TN]),
            )

            def cv2_pair(pi):
                i0 = 2 * pi
                for h in range(2):
                    po = psum.tile([CH, 2, 512], f32, tag="ps")
                    if tune["cv2co"]:
                        for c, rhs_fn in CV2_CHUNKS:
                            wt = w2_sb[:, c * 256 + h * 128:
                                       c * 256 + h * 128 + 128]
                            for ii in range(2):
                                nc.tensor.matmul(po[:, ii, 0:TN], wt,
                                                 rhs_fn(i0 + ii),
                                                 start=(c == 0), stop=(c == 2))
                    else:
                        for ii in range(2):
                            for c, rhs_fn in CV2_CHUNKS:
                                nc.tensor.matmul(
                                    po[:, ii, 0:TN],
                                    w2_sb[:, c * 256 + h * 128:
                                          c * 256 + h * 128 + 128],
                                    rhs_fn(i0 + ii),
                                    start=(c == 0), stop=(c == 2))
                    ot = opool.tile([CH, 2 * TN], ydt, tag="ot")
                    act_silu(ot[:].rearrange("p (g c) -> p g c", g=2),
                             po[:, :, 0:TN], b2_sb[:, h:h + 1])
                    nc.sync.dma_start(y_d[b, h, :, pi * 800:(pi + 1) * 800], ot[:])

            def cv2_group(pg):
                # chunk-outer over 2 pairs: each w2 slice loads once per
                # 4 matmuls
                for h in range(2):
                    poA = psum.tile([CH, 2, 512], f32, tag="ps", name="poA")
                    poB = psum.tile([CH, 2, 512], f32, tag="ps", name="poB")
                    for c, rhs_fn in CV2_CHUNKS:
                        wt = w2_sb[:, c * 256 + h * 128: c * 256 + h * 128 + 128]
                        for po, pi in ((poA, 2 * pg), (poB, 2 * pg + 1)):
                            i0 = 2 * pi
                            for ii in range(2):
                                nc.tensor.matmul(po[:, ii, 0:TN], wt,
                                                 rhs_fn(i0 + ii),
                                                 start=(c == 0), stop=(c == 2))
                    for po, pi in ((poA, 2 * pg), (poB, 2 * pg + 1)):
                        ot = opool.tile([CH, 2 * TN], ydt, tag="ot")
                        act_silu(ot[:].rearrange("p (g c) -> p g c", g=2),
                                 po[:, :, 0:TN], b2_sb[:, h:h + 1])
                        nc.sync.dma_start(y_d[b, h, :, pi * 800:(pi + 1) * 800],
                                          ot[:])

            def moe_combine(ps2, pi):
                rt = rpool.tile([CH, 2 * TN], dmm if tune["shbf16"] else f32,
                                tag="rt")
                act_silu(rt[:].rearrange("p (g c) -> p g c", g=2),
                         ps2[:, :, 0:TN], bS[:])
                nc.vector.scalar_tensor_tensor(
                    moe_sb[:, pi * 800:(pi + 1) * 800], rt[:], sc[:, NE:NE + 1],
                    sh_sb[:, pi * 800:(pi + 1) * 800], op0=OP.mult, op1=OP.add)

            if tune["g2"]:
                for pg in range(NP // 2):
                    psA, psB = conv_group(wS, pg)
                    moe_combine(psA, 2 * pg)
                    moe_combine(psB, 2 * pg + 1)
                    if pg > 0:
                        cv2_group(pg - 1)
                cv2_group(NP // 2 - 1)
            else:
                for pi in range(NP):
                    ps2 = psum.tile([CH, 2, 512], f32, tag="ps")
                    conv_pair(ps2, wS, pi)
                    moe_combine(ps2, pi)
                    if pi > 0:
                        cv2_pair(pi - 1)
                cv2_pair(NP - 1)

    if reps == 1:
        _body()
    else:
        # HW timing mode: repeat the whole workload in a hardware loop
        # (same instruction count / compile cost; R x device work).
        with tc.For_i(0, reps, 1):
            _body()
    if internal_io:
        # tiny external output so the (otherwise internal-IO) program is not
        # dead-code eliminated; depends on the looped work via y.
        ydig_d = nc.dram_tensor("ydig", [CH, 4], ydt,
                                kind="ExternalOutput").ap()
        ydig_t = opool.tile([CH, 4], ydt, name="ydig_t")
        nc.sync.dma_start(ydig_t[:], y_d[0, 0, :, 0:4])
        nc.sync.dma_start(ydig_d, ydig_t[:])


def _dedup_ldweights(nc):
    """Remove back-to-back redundant PE weight loads.

    The Tile scheduler emits one InstLdweights per matmul even when
    consecutive matmuls use identical stationary weights; on TRN2 the PE
    retains the stationary operand across matmuls, and LDW+MM serialize, so
    each redundant load costs ~real time. Drop an LDW when the PE weight
    state provably already matches: same lowered weights AP, no intervening
    weight-clobbering instruction (fp32/fp32r matmuls self-load), and the
    LDW carries no semaphore waits/updates (so sync placement is unchanged).
    """
    f32s = (mybir.dt.float32, mybir.dt.float32r)
    name_map = {}
    removed = 0
    for blk in nc.m.functions[0].blocks:
        il = blk.instructions
        cur_sig = None
        cur_name = None
        keep = []
        changed = False
        for inst in il:
            if isinstance(inst, mybir.InstLdweights):
                sig = (str(inst.ins[0]), str(inst.perf_mode),
                       str(inst.is_transpose), str(inst.tile_position))
                if (sig == cur_sig and not inst.has_wait()
                        and not inst.has_update()):
                    name_map[inst.name] = cur_name
                    removed += 1
                    changed = True
                    continue
                cur_sig = sig
                cur_name = inst.name
            elif isinstance(inst, mybir.InstMatmult):
                if inst.ins[1].dtype in f32s or inst.is_transpose:
                    cur_sig = None
                    cur_name = None
            keep.append(inst)
        if changed:
            il[:] = keep
    if name_map:
        for blk in nc.m.functions[0].blocks:
            for inst in blk.instructions:
                inst.remap_dependency_names(name_map)
    return removed


def build(reps=1, sim_compat=False, tune=None, internal_io=False):
    from contextlib import ExitStack
    nc = bacc.Bacc("TRN2", target_bir_lowering=False, debug=False,
                   num_devices=NCORES)
    with tile.TileContext(nc) as tc:
        with ExitStack() as ctx:
            _emit(nc, tc, ctx, reps=reps, sim_compat=sim_compat, tune=tune,
                  internal_io=internal_io)
    if (tune or {}).get("dedup"):
        nc.move_matmul_waits_to_ldweights()
        n = _dedup_ldweights(nc)
        print(f"dedup_ldweights: removed {n}")
    nc.compile()
    return nc


def round_f32r(a):
    """Round fp32 to the PE's fp32r format: 11 explicit mantissa bits
    (round-to-nearest-even), low 12 bits zero. The result is both a valid
    fp32 value and a valid fp32r bit pattern."""
    a = np.ascontiguousarray(np.asarray(a, np.float32))
    bits = a.view(np.uint32).astype(np.uint64)
    lsb = (bits >> 12) & 1
    r = (bits + 0x7FF + lsb) & 0xFFFFF000
    return r.astype(np.uint32).view(np.float32)


def marshal_inputs(x, w1, b1, wr, br, ws, bs, we, be, w2, b2, use_bf16=False):
    """Host-side (tiny) weight re-layouts into matmul-friendly forms."""
    asf = lambda a: np.ascontiguousarray(np.asarray(a, dtype=np.float32))
    if use_bf16:
        import ml_dtypes
        cvt = lambda a: np.ascontiguousarray(
            np.asarray(a, np.float32).astype(ml_dtypes.bfloat16))
    else:
        cvt = round_f32r
    x = cvt(x)
    w1t = asf(np.asarray(w1, np.float32).reshape(2 * CH, C1).T.reshape(2, CH, 2 * CH))
    b1r = asf(np.asarray(b1, np.float32).reshape(2, CH))
    wrs = asf(np.asarray(wr, np.float32) / NPIX)
    brr = asf(np.tile(np.asarray(br, np.float32).reshape(1, NE), (CH, 1)))
    wst = asf(np.asarray(ws, np.float32).transpose(1, 2, 3, 0).reshape(CH, 9 * CH))
    bsr = asf(np.asarray(bs, np.float32).reshape(CH, 1))
    wet = asf(np.asarray(we, np.float32).transpose(0, 2, 3, 4, 1).reshape(NE, CH, 9 * CH))
    ber = asf(np.asarray(be, np.float32).T)
    w2t = asf(np.asarray(w2, np.float32).reshape(C2, 3 * CH).T.reshape(3, CH, C2))
    b2r = asf(np.asarray(b2, np.float32).reshape(2, CH))
    w1t = cvt(w1t)
    wst = cvt(wst)
    wet = round_f32r(wet) if not use_bf16 else wet
    w2t = cvt(w2t)
    shared = dict(w1t=w1t, b1r=b1r, wrs=wrs, brr=brr, wst=wst, bsr=bsr,
                  wet=wet, ber=ber, w2t=w2t, b2r=b2r)
    xc = x.reshape(NCORES, BPC, 2, CH, NPIX)
    in_maps = [dict(shared, x=np.ascontiguousarray(xc[c])) for c in range(NCORES)]
    return in_maps


_CACHE = {}

# Module-level tune overrides (picked by benchmarking); kernel() and
# test.py's timing builds both honor this.
TUNE = {"bf16": True, "dedup": True, "adefer": True,
        "ybf16": True, "shbf16": True, "gsrouter": True,
        "fpdouble": True, "adouble": True, "psbufs": 4}


def _get_nc():
    if "nc" not in _CACHE:
        _CACHE["nc"] = build(reps=1, tune=TUNE)
    return _CACHE["nc"]


def _get_runner():
    """Build the sharded PJRT callable once (mirrors
    bass2jax.run_bass_via_pjrt's multi-core path) so repeat kernel() calls
    skip the jax retrace/compile."""
    if "runner" in _CACHE:
        return _CACHE["runner"]
    import jax
    from jax.experimental.shard_map import shard_map
    from jax.sharding import Mesh, PartitionSpec
    from concourse import bass2jax

    nc = _get_nc()
    bass2jax.install_neuronx_cc_hook()
    part_name = nc.partition_id_tensor.name if nc.partition_id_tensor else None
    in_names, out_names, out_avals = [], [], []
    for alloc in nc.m.functions[0].allocations:
        if not isinstance(alloc, mybir.MemoryLocationSet):
            continue
        name = alloc.memorylocations[0].name
        if alloc.kind == "ExternalInput":
            if name != part_name:
                in_names.append(name)
        elif alloc.kind == "ExternalOutput":
            out_names.append(name)
            out_avals.append(jax.core.ShapedArray(
                tuple(alloc.tensor_shape), mybir.dt.np(alloc.dtype)))
    assert nc.dbg_addr is None
    n_params = len(in_names)
    all_in = in_names + out_names  # zero buffers donated as outputs
    if part_name is not None:
        all_in = all_in + [part_name]

    def _body(*args):
        operands = list(args)
        if part_name is not None:
            operands.append(bass2jax.partition_id_tensor())
        outs = bass2jax._bass_exec_p.bind(
            *operands, out_avals=tuple(out_avals), in_names=tuple(all_in),
            out_names=tuple(out_names), lowering_input_output_aliases=(),
            sim_require_finite=True, sim_require_nnan=True, nc=nc)
        return tuple(outs)

    devices = jax.devices()[:NCORES]
    mesh = Mesh(np.asarray(devices), ("core",))
    nio = n_params + len(out_names)
    sharded = jax.jit(
        shard_map(_body, mesh=mesh, in_specs=(PartitionSpec("core"),) * nio,
                  out_specs=(PartitionSpec("core"),) * len(out_names),
                  check_rep=False),
        donate_argnums=tuple(range(n_params, nio)), keep_unused=True)
    _CACHE["runner"] = (sharded, in_names, out_names, out_avals)
    return _CACHE["runner"]


def kernel(x, w1, b1, wr, br, ws, bs, we, be, w2, b2):
    in_maps = marshal_inputs(x, w1, b1, wr, br, ws, bs, we, be, w2, b2,
                             use_bf16=bool(TUNE.get("bf16")))
    sharded, in_names, out_names, out_avals = _get_runner()
    concat_in = [
        np.concatenate([in_maps[c][name] for c in range(NCORES)], axis=0)
        for name in in_names
    ]
    concat_zeros = [
        np.zeros((NCORES * a.shape[0], *a.shape[1:]), a.dtype) for a in out_avals
    ]
    out_arrs = sharded(*concat_in, *concat_zeros)
    y = np.asarray(out_arrs[out_names.index("y")])
    return np.ascontiguousarray(
        y.astype(np.float32, copy=False).reshape(B, C2, H, W))

